# revision 13
# baseline (speedup 1.0000x reference)
"""Trainium2 Bass kernel for nn_GCN_3 (gnn_message_passing).

Strategy (8 NeuronCores, SPMD single NEFF):
  - 2 cores per batch (pair p = [2b, 2b+1]); within a pair the padded node
    dimension (1568 -> 1664 = 13x128) is split 896 (tiles 0..6, even core) /
    768 (tiles 7..12, odd core); all SBUF/DRAM buffers are sized 896 rows so
    the program is identical on every core (per-core behavior differs only
    through input *values* and collective rank order).
  - Exact algebraic simplification: column sums of the normalized adjacency
    are exactly 2, so D^-1/2 A D^-1/2 = 0.5*(sq/colsum + I). The kernel
    builds lapST[j, i] = 0.5*sq[j,i]/colsum[j] + 0.5*I directly (sq is
    symmetric), which is the lhsT-side operand for every graph-conv matmul.
  - Chain of layouts avoids all transposes:
      xT (host) --fc1--> hT[d,j] --s--> s[:,own] --scale--> lapST[j,own]
      v[j,c] x lapST -> aggT[c,own] x wT -> y[own,d] -> LN/LReLU -> v rows
  - Pair exchanges ride 2-rank AllGathers (fixed block offsets), split in 2
    chunks to overlap; the tiny colsum partial sum also rides a 2-rank AG.
  - Head: upsample+1x1conv folded into one host matrix M (built with
    jax.image.resize on CPU); deconv(stride 3, k3, pad 1) = 9 phase-wise
    1x1 convs, output channels split 512/512 across the pair.
All matmul/compute f32 (PE multiplies at FP22, accumulates FP32).
"""
import numpy as np

import concourse.bass as bass
import concourse.mybir as mybir
import concourse.tile as tile
from concourse import bacc
from concourse.bass_utils import run_bass_kernel_spmd

F32 = mybir.dt.float32
AX = mybir.AxisListType
OP = mybir.AluOpType
ACT = mybir.ActivationFunctionType

NODES = 1568
NP_ = 1664          # padded nodes = 13*128
C = 2048
NCORES = 8
OWN = 896           # uniform per-core row-buffer size (7 tiles)
KT = 13             # node k-tiles
CT = 16             # channel k-tiles
GROUPS = [[0, 1], [2, 3], [4, 5], [6, 7]]
# node-tile -> (which v-AG chunk, row offset inside that chunk's output)
#   chunk1: in rows 0:512  -> out [1024, C]: blocks [A 0:512 | B 0:512]
#   chunk2: in rows 512:896 -> out [768, C]: blocks [A 512:896 | B 512:896]
V_SRC = {0: (0, 0), 1: (0, 128), 2: (0, 256), 3: (0, 384),
         7: (0, 512), 8: (0, 640), 9: (0, 768), 10: (0, 896),
         4: (1, 0), 5: (1, 128), 6: (1, 256),
         11: (1, 384), 12: (1, 512)}
# deconv phase tables: residue -> (kernel tap, a0, count, out start)
PHASE = {0: (1, 0, 28, 0), 1: (0, 0, 27, 1), 2: (2, 1, 27, 2)}

_CACHE = {}


def _build_nc(gb_trivial, cb):
    nc = bacc.Bacc(None, target_bir_lowering=False, num_devices=NCORES)

    x_pad = nc.dram_tensor("x_pad", [NP_, C], F32, kind="ExternalInput")
    xT_own = nc.dram_tensor("xT_own", [C, OWN], F32, kind="ExternalInput")
    wfc1T = nc.dram_tensor("wfc1T", [C, C], F32, kind="ExternalInput")
    wT = [nc.dram_tensor(f"w{i}T", [C, C], F32, kind="ExternalInput")
          for i in (1, 2, 3)]
    gv = [nc.dram_tensor(f"g{i}", [C], F32, kind="ExternalInput")
          for i in (1, 2, 3)]
    bv = [nc.dram_tensor(f"b{i}", [C], F32, kind="ExternalInput")
          for i in (1, 2, 3)]
    eye_add = nc.dram_tensor("eye_add", [KT, 128, OWN], F32,
                             kind="ExternalInput")
    MT_in = nc.dram_tensor("MT", [NP_, 784], F32, kind="ExternalInput")
    wkT_in = nc.dram_tensor("wkT", [3, 3, C, 512], F32, kind="ExternalInput")
    # phase-major deconv output (contiguous DMA); host de-interleaves
    out_ph = nc.dram_tensor("out_ph", [9, 512, 28, 28], F32,
                            kind="ExternalOutput")

    with tile.TileContext(nc) as tc:
        with (
            tc.tile_pool(name="work", bufs=1) as wk,
            tc.tile_pool(name="st", bufs=2) as st,
            tc.tile_pool(name="psum", bufs=1, space="PSUM") as psp,
            tc.tile_pool(name="dram", bufs=1, space="DRAM") as dr,
        ):
            def ps(i):
                return psp.tile([128, 512], F32, name=f"pp{i}", tag=f"pp{i}")

            # persistent slot helpers (tags reused across phases)
            def slot_a(i, n=OWN):
                return wk.tile([128, OWN], F32, name=f"a{i}", tag=f"a{i}")[:, :n]

            def slot_b(i, n=OWN):
                return wk.tile([128, OWN], F32, name=f"b{i}", tag=f"b{i}")[:, :n]

            # ---------------- internal DRAM ----------------
            ht_in_a = dr.tile([1024, OWN], F32, name="ht_in_a")
            ht_in_b = dr.tile([1024, OWN], F32, name="ht_in_b")
            ht_out_a = dr.tile([2048, OWN], F32, name="ht_out_a")
            ht_out_b = dr.tile([2048, OWN], F32, name="ht_out_b")
            cs_in = dr.tile([1, NP_], F32, name="cs_in")
            cs_out = dr.tile([2, NP_], F32, name="cs_out")
            v_in1 = [dr.tile([512, C], F32, name=f"vi1_{l}") for l in range(3)]
            v_in2 = [dr.tile([384, C], F32, name=f"vi2_{l}") for l in range(3)]
            v_o1 = [dr.tile([1024, C], F32, name=f"vo1_{l}") for l in range(3)]
            v_o2 = [dr.tile([768, C], F32, name=f"vo2_{l}") for l in range(3)]
            y_dram = dr.tile([OWN, C], F32, name="y_dram")

            def ag(in_ap, out_ap):
                nc.gpsimd.collective_compute(
                    "AllGather", OP.bypass, replica_groups=GROUPS,
                    ins=[in_ap], outs=[out_ap])

            # ============ Phase 1: fc1  hT[d, own_j] ============
            xt_sb = [slot_a(i) for i in range(CT)]
            for i in range(CT):
                nc.sync.dma_start(xt_sb[i][:], xT_own[i * 128:(i + 1) * 128, :])
            for mc in range(CT):
                p0, p1 = ps(0), ps(1)
                pp = (p0[:, :448], p1[:, :448])
                for kt in range(CT):
                    wtile = st.tile([128, 128], F32, name="wl", tag="wl")
                    nc.sync.dma_start(
                        wtile[:],
                        wfc1T[kt * 128:(kt + 1) * 128,
                              mc * 128:(mc + 1) * 128])
                    for nch in range(2):
                        nc.tensor.matmul(
                            pp[nch], wtile[:],
                            xt_sb[kt][:, nch * 448:(nch + 1) * 448],
                            start=(kt == 0), stop=(kt == CT - 1))
                ev = st.tile([128, OWN], F32, name="ev", tag="ev")
                for nch in range(2):
                    nc.scalar.copy(ev[:, nch * 448:(nch + 1) * 448], pp[nch])
                dst = ht_in_a if mc < 8 else ht_in_b
                ro = (mc % 8) * 128
                nc.sync.dma_start(dst[ro:ro + 128, :], ev[:])
            ag(ht_in_a[:], ht_out_a[:])
            ag(ht_in_b[:], ht_out_b[:])

            # ============ Phase 2: s[:, own], colsum, lapST ============
            # lhsT tile source for s: hT_full[d-kt, j-mt]
            def ht_tile(kt, mt):
                buf = ht_out_a if kt < 8 else ht_out_b
                d0 = (kt % 8) * 128
                if mt < 7:
                    return buf[d0:d0 + 128, mt * 128:(mt + 1) * 128]
                return buf[1024 + d0:1024 + d0 + 128,
                           (mt - 7) * 128:(mt - 6) * 128]

            # rhs: own hT columns = re-load own fc1 output from ht_in
            hto_sb = [slot_a(i) for i in range(CT)]
            for i in range(CT):
                src = ht_in_a if i < 8 else ht_in_b
                ro = (i % 8) * 128
                nc.sync.dma_start(hto_sb[i][:], src[ro:ro + 128, :])

            cs_sb = wk.tile([128, KT], F32, name="cs_sb", tag="cs_sb")
            lap_sb = [slot_b(i) for i in range(KT)]
            for mt in range(KT):
                p0, p1 = ps(2), ps(3)
                pp = (p0[:, :448], p1[:, :448])
                for kt in range(CT):
                    lt = st.tile([128, 128], F32, name="lt", tag="lt")
                    nc.sync.dma_start(lt[:], ht_tile(kt, mt))
                    for nch in range(2):
                        nc.tensor.matmul(
                            pp[nch], lt[:],
                            hto_sb[kt][:, nch * 448:(nch + 1) * 448],
                            start=(kt == 0), stop=(kt == CT - 1))
                for nch in range(2):
                    nc.scalar.square(
                        lap_sb[mt][:, nch * 448:(nch + 1) * 448], pp[nch])
                nc.vector.tensor_reduce(
                    cs_sb[:, mt:mt + 1], lap_sb[mt][:], AX.X, OP.add)
            # colsum partial -> AG(pair) -> scale
            nc.sync.dma_start(
                cs_in[0].rearrange("(t p) -> p t", p=128), cs_sb[:])
            ag(cs_in[:], cs_out[:])
            cs0 = wk.tile([128, KT], F32, name="cs0", tag="cs0")
            cs1 = wk.tile([128, KT], F32, name="cs1", tag="cs1")
            nc.sync.dma_start(cs0[:], cs_out[0].rearrange("(t p) -> p t", p=128))
            nc.sync.dma_start(cs1[:], cs_out[1].rearrange("(t p) -> p t", p=128))
            nc.vector.tensor_tensor(cs0[:], cs0[:], cs1[:], OP.add)
            nc.vector.tensor_scalar_add(cs0[:], cs0[:], 1e-20)
            nc.vector.reciprocal(cs0[:], cs0[:])
            nc.vector.tensor_scalar_mul(cs0[:], cs0[:], 0.5)
            for mt in range(KT):
                nc.vector.tensor_scalar_mul(
                    lap_sb[mt][:], lap_sb[mt][:], cs0[:, mt:mt + 1])
                ey = st.tile([128, OWN], F32, name="ey", tag="ey")
                nc.sync.dma_start(ey[:], eye_add[mt])
                nc.vector.tensor_tensor(lap_sb[mt][:], lap_sb[mt][:], ey[:],
                                        OP.add)

            # ============ Phase 3: 3 graph-conv layers ============
            def v_tile(layer, kt, c0, cw):
                """lhsT tile [128, cw] of v (node rows kt*128, cols c0:c0+cw)."""
                if layer == 0:
                    return x_pad[kt * 128:(kt + 1) * 128, c0:c0 + cw]
                ch, ro = V_SRC[kt]
                buf = (v_o1 if ch == 0 else v_o2)[layer - 1]
                return buf[ro:ro + 128, c0:c0 + cw]

            for layer in range(3):
                # --- aggT[c, own] = sum_j v[j, c] * lapST[j, own] ---
                agg_sb = [slot_a(i) for i in range(CT)]
                for mp in range(8):           # pairs of output c-tiles
                    pq = [ps(i) for i in range(4)]
                    pp = [q[:, :448] for q in pq]
                    for ki, kt in enumerate(V_SRC):  # insertion order
                        lt = st.tile([128, 256], F32, name="lt2", tag="lt2")
                        nc.sync.dma_start(
                            lt[:], v_tile(layer, kt, mp * 256, 256))
                        for mi in range(2):
                            for nch in range(2):
                                nc.tensor.matmul(
                                    pp[mi * 2 + nch],
                                    lt[:, mi * 128:(mi + 1) * 128],
                                    lap_sb[kt][:, nch * 448:(nch + 1) * 448],
                                    start=(ki == 0), stop=(ki == KT - 1))
                    for mi in range(2):
                        for nch in range(2):
                            nc.scalar.copy(
                                agg_sb[mp * 2 + mi][:, nch * 448:(nch + 1) * 448],
                                pp[mi * 2 + nch])
                # --- y[own, d] = aggT.T @ wT ---
                for nch in range(4):
                    pq = [ps(i) for i in range(7)]
                    for kt in range(CT):
                        wtile = st.tile([128, 512], F32, name="wl2", tag="wl2")
                        nc.sync.dma_start(
                            wtile[:],
                            wT[layer][kt * 128:(kt + 1) * 128,
                                      nch * 512:(nch + 1) * 512])
                        for mt in range(7):
                            nc.tensor.matmul(
                                pq[mt], agg_sb[kt][:, mt * 128:(mt + 1) * 128],
                                wtile[:],
                                start=(kt == 0), stop=(kt == CT - 1))
                    for mt in range(7):
                        ev = st.tile([128, 512], F32, name="ev2", tag="ev2")
                        nc.scalar.copy(ev[:], pq[mt])
                        nc.sync.dma_start(
                            y_dram[mt * 128:(mt + 1) * 128,
                                   nch * 512:(nch + 1) * 512], ev[:])
                # --- LayerNorm + LeakyReLU per row-tile; write cc inputs ---
                for mt in range(7):
                    yt = st.tile([128, C], F32, name="yt", tag="yt")
                    nc.sync.dma_start(yt[:], y_dram[mt * 128:(mt + 1) * 128, :])
                    nm = st.tile([128, 1], F32, name="nm", tag="nm")
                    nc.vector.tensor_reduce(nm[:], yt[:], AX.X, OP.add,
                                            negate=True)
                    nc.vector.tensor_scalar_mul(nm[:], nm[:], 1.0 / C)
                    ssq_c = st.tile([128, 4], F32, name="ssq_c", tag="ssq_c")
                    for q in range(4):
                        sqd = st.tile([128, 512], F32, name="sqd", tag="sqd")
                        nc.scalar.activation(
                            sqd[:], yt[:, q * 512:(q + 1) * 512], ACT.Square,
                            accum_out=ssq_c[:, q:q + 1])
                    ssq = st.tile([128, 1], F32, name="ssq", tag="ssq")
                    nc.vector.tensor_reduce(ssq[:], ssq_c[:], AX.X, OP.add)
                    var = st.tile([128, 1], F32, name="var", tag="var")
                    m2 = st.tile([128, 1], F32, name="m2", tag="m2")
                    nc.vector.tensor_tensor(m2[:], nm[:], nm[:], OP.mult)
                    nc.vector.tensor_scalar(var[:], ssq[:], 1.0 / C, 1e-5,
                                            OP.mult, OP.add)
                    nc.vector.tensor_tensor(var[:], var[:], m2[:],
                                            OP.subtract)
                    std = st.tile([128, 1], F32, name="std", tag="std")
                    nc.scalar.sqrt(std[:], var[:])
                    rstd = st.tile([128, 1], F32, name="rstd", tag="rstd")
                    nc.vector.reciprocal(rstd[:], std[:])
                    nb = st.tile([128, 1], F32, name="nb", tag="nb")
                    nc.vector.tensor_tensor(nb[:], nm[:], rstd[:], OP.mult)
                    zt = st.tile([128, C], F32, name="zt", tag="zt")
                    nc.scalar.activation(zt[:], yt[:], ACT.Identity,
                                         bias=nb[:], scale=rstd[:])
                    if not gb_trivial:
                        gs = st.tile([1, C], F32, name="gs", tag="gs")
                        bs = st.tile([1, C], F32, name="bs", tag="bs")
                        nc.sync.dma_start(gs[:], gv[layer][None, :])
                        nc.sync.dma_start(bs[:], bv[layer][None, :])
                        nc.vector.tensor_tensor(
                            zt[:], zt[:], gs[:].to_broadcast(zt.shape), OP.mult)
                        nc.vector.tensor_tensor(
                            zt[:], zt[:], bs[:].to_broadcast(zt.shape), OP.add)
                    z1 = st.tile([128, C], F32, name="z1", tag="z1")
                    nc.vector.tensor_scalar_mul(z1[:], zt[:], 0.1)
                    nc.vector.tensor_tensor(z1[:], zt[:], z1[:], OP.max)
                    if mt < 4:
                        nc.sync.dma_start(
                            v_in1[layer][mt * 128:(mt + 1) * 128, :], z1[:])
                    else:
                        nc.sync.dma_start(
                            v_in2[layer][(mt - 4) * 128:(mt - 3) * 128, :],
                            z1[:])
                ag(v_in1[layer][:], v_o1[layer][:])
                ag(v_in2[layer][:], v_o2[layer][:])

            # ============ Phase 4: head (upsample fold + deconv) ============
            mt_sb = [slot_b(i, 784) for i in range(KT)]
            for i in range(KT):
                nc.sync.dma_start(mt_sb[i][:], MT_in[i * 128:(i + 1) * 128, :])
            up_sb = [slot_a(i, 784) for i in range(CT)]
            for mp in range(8):
                pq = [ps(i) for i in range(4)]
                pp = [q[:, :392] for q in pq]
                for ki, kt in enumerate(V_SRC):
                    lt = st.tile([128, 256], F32, name="lt2", tag="lt2")
                    nc.sync.dma_start(lt[:], v_tile(3, kt, mp * 256, 256))
                    for mi in range(2):
                        for nch in range(2):
                            nc.tensor.matmul(
                                pp[mi * 2 + nch],
                                lt[:, mi * 128:(mi + 1) * 128],
                                mt_sb[kt][:, nch * 392:(nch + 1) * 392],
                                start=(ki == 0), stop=(ki == KT - 1))
                for mi in range(2):
                    for nch in range(2):
                        nc.scalar.add(
                            up_sb[mp * 2 + mi][:, nch * 392:(nch + 1) * 392],
                            pp[mi * 2 + nch], cb)
            # deconv: 9 phases of 1x1 convs over c
            for r in range(3):
                di, a0, na, i0 = PHASE[r]
                for t in range(3):
                    dj, b0, nb_, j0 = PHASE[t]
                    ach = [(0, (na + 1) // 2), ((na + 1) // 2,
                                               na - (na + 1) // 2)]
                    for op_ in range(2):       # output-channel tile pairs
                        pq = [ps(i) for i in range(4)]
                        for kt in range(CT):
                            wkt = st.tile([128, 256], F32, name="wk",
                                          tag="wk")
                            nc.sync.dma_start(
                                wkt[:],
                                wkT_in[di, dj, kt * 128:(kt + 1) * 128,
                                       op_ * 256:(op_ + 1) * 256])
                            for mi in range(2):
                                for nch in range(2):
                                    ao, an = ach[nch]
                                    rhs = up_sb[kt].rearrange(
                                        "p (a b) -> p a b", b=28)[
                                        :, a0 + ao:a0 + ao + an,
                                        b0:b0 + nb_]
                                    nc.tensor.matmul(
                                        pq[mi * 2 + nch][:, :an * nb_],
                                        wkt[:, mi * 128:(mi + 1) * 128],
                                        rhs,
                                        start=(kt == 0), stop=(kt == CT - 1))
                        for mi in range(2):
                            oc0 = (op_ * 2 + mi) * 128
                            for nch in range(2):
                                ao, an = ach[nch]
                                ev = st.tile([128, 512], F32, name="ev3",
                                             tag="ev3")
                                nc.vector.tensor_copy(
                                    ev[:, :an * nb_],
                                    pq[mi * 2 + nch][:, :an * nb_])
                                nc.sync.dma_start(
                                    out_ph[r * 3 + t, oc0:oc0 + 128,
                                           ao:ao + an, 0:nb_],
                                    ev[:, :an * nb_])

    nc.compile()
    return nc


def kernel(**inputs):
    import jax
    x = np.ascontiguousarray(np.asarray(inputs["x"], np.float32))
    B = x.shape[0]
    conv1_w = np.asarray(inputs["conv1_w"], np.float32)
    cb = float(np.asarray(inputs["conv1_b"]).reshape(-1)[0])
    g_triv = all(
        np.all(np.asarray(inputs[f"g{i}"]) == 1) and
        np.all(np.asarray(inputs[f"b{i}"]) == 0) for i in (1, 2, 3))

    with jax.default_device(jax.devices("cpu")[0]):
        U = np.asarray(jax.image.resize(
            np.eye(14, dtype=np.float32), (28, 14), "bilinear"), np.float32)
    M = np.einsum("f,ai,bj->abfij", conv1_w, U, U).reshape(784, NODES)
    MT = np.zeros((NP_, 784), np.float32)
    MT[:NODES] = M.T
    wk_flip = np.flip(np.asarray(inputs["deconv_w"], np.float32), (2, 3))
    wkT_full = np.ascontiguousarray(wk_flip.transpose(2, 3, 0, 1))  # [3,3,C,1024]

    eye = [np.zeros((KT, 128, OWN), np.float32) for _ in range(2)]
    for j in range(NODES):
        t, p = j // 128, j % 128
        if j < OWN:
            eye[0][t, p, j] = 0.5
        else:
            eye[1][t, p, j - OWN] = 0.5

    wfc1T = np.ascontiguousarray(np.asarray(inputs["w_fc1"], np.float32).T)
    wTs = [np.ascontiguousarray(np.asarray(inputs[f"w{i}"], np.float32).T)
           for i in (1, 2, 3)]

    key = ("nc", g_triv, round(cb, 9))
    if key not in _CACHE:
        _CACHE[key] = _build_nc(g_triv, cb)
    nc = _CACHE[key]

    in_maps = []
    for c in range(NCORES):
        b, par = c // 2, c % 2
        xp = np.zeros((NP_, C), np.float32)
        xp[:NODES] = x[b]
        xT = np.zeros((C, NP_), np.float32)
        xT[:, :NODES] = x[b].T
        xto = np.ascontiguousarray(
            xT[:, :OWN] if par == 0 else
            np.pad(xT[:, OWN:], ((0, 0), (0, 2 * OWN - NP_))))
        m = {
            "x_pad": xp, "xT_own": xto, "wfc1T": wfc1T,
            "w1T": wTs[0], "w2T": wTs[1], "w3T": wTs[2],
            "g1": np.asarray(inputs["g1"], np.float32),
            "g2": np.asarray(inputs["g2"], np.float32),
            "g3": np.asarray(inputs["g3"], np.float32),
            "b1": np.asarray(inputs["b1"], np.float32),
            "b2": np.asarray(inputs["b2"], np.float32),
            "b3": np.asarray(inputs["b3"], np.float32),
            "eye_add": eye[par],
            "MT": MT,
            "wkT": np.ascontiguousarray(
                wkT_full[:, :, :, par * 512:(par + 1) * 512]),
        }
        in_maps.append(m)

    res = run_bass_kernel_spmd(nc, in_maps, core_ids=list(range(NCORES)))
    out = np.zeros((B, 1024, 82, 82), np.float32)
    for c in range(NCORES):
        b, par = c // 2, c % 2
        oph = res.results[c]["out_ph"]  # [9, 512, 28, 28]
        for r in range(3):
            _, _, na, i0 = PHASE[r]
            for t in range(3):
                _, _, nb_, j0 = PHASE[t]
                out[b, par * 512:(par + 1) * 512,
                    i0:i0 + 3 * na:3, j0:j0 + 3 * nb_:3] = \
                    oph[r * 3 + t, :, :na, :nb_]
    out += np.asarray(inputs["deconv_b"], np.float32)[None, :, None, None]
    return out
